# revision 33
# baseline (speedup 1.0000x reference)
"""Causal multi-head self-attention (B=2, T=2048, D=1024, H=16) on 8 TRN2
NeuronCores.

Sharding (Megatron-style, hardcoded): core = 4*b + g where b in {0,1} is the
batch and g in {0..3} a group of 4 heads. Each core computes Q/K/V projections
for its head group from x[b], fused causal attention for those 4 heads, and a
partial output projection against its 256-column slice of Wo. The host sums
the 4 partial outputs per batch (the all-reduce after out_proj).

v3 design notes:
 - All matmul operands bf16 (PSUM accumulation f32): halves input DMA bytes,
   keeps matmuls at 1 row/cycle at any free size, enabling exact-causal
   N-slicing of diagonal score/PV tiles.
 - Input DMAs sized >=256KB (ct-major Wq/Wk layout) split across the scalar
   and sync HWDGE queues so the first projection matmul starts ~7us in.
 - Scores transposed (S^T[k, q]) per head pair on disjoint PE row groups
   (concurrent when warm); exp on Scalar writes bf16; diagonal 128-col band
   masked post-exp with gpsimd.affine_select.
 - Row sums via a ones-column appended to V (row 64 of the PV accumulator).
   Normalization is DMA-free: Z row reshaped via DVE 32x32 stream transpose,
   multi-lane reciprocal on the strided view, transpose back, gpsimd
   partition_broadcast, divide folded into the bf16 `at` write.
 - Software pipelining: PV(kti) emitted after scores(kti+1); Q/K/V units of
   chunk qc+1 and out-projection units of earlier chunks interleave into the
   kti loop as PE filler (keeps HAM at K=8/8). Norm stages run as deferred
   work spread over subsequent iterations (crossing chunk boundaries) so no
   engine FIFO blocks on them.
"""

from collections import deque

import numpy as np
import ml_dtypes

import concourse.bass as bass
import concourse.tile as tile
from concourse import bacc, mybir
from concourse.bass_utils import run_bass_kernel_spmd

B, T, D, H, DH = 2, 2048, 1024, 16, 64
HPC = 4  # heads per core
GC = 256  # projection columns per core (HPC * DH)
N_CORES = 8
F32 = mybir.dt.float32
BF16 = mybir.dt.bfloat16
EXP = mybir.ActivationFunctionType.Exp

_CACHE = {}


def _build():
    nc = bacc.Bacc(
        "TRN2", target_bir_lowering=False, debug=False, num_devices=N_CORES
    )
    # Pre-swizzled inputs (host does the transposes + bf16 cast):
    #   xs[p, tc, dt, t] = x[b, tc*512+t, dt*128+p]
    #   wq/wk[p, ct, dt, c] = W[g*256 + ct*128 + c, dt*128+p]
    #   wv[p, dt, c] = Wv[g*256+c, dt*128+p]
    #   wo[p, ct, n] = Wo[n, g*256 + ct*128 + p]
    xs = nc.dram_tensor("xs", [128, 4, 8, 512], BF16, kind="ExternalInput").ap()
    wqs = nc.dram_tensor("wqs", [128, 2, 8, 128], BF16, kind="ExternalInput").ap()
    wks = nc.dram_tensor("wks", [128, 2, 8, 128], BF16, kind="ExternalInput").ap()
    wvs = nc.dram_tensor("wvs", [128, 8, GC], BF16, kind="ExternalInput").ap()
    wos = nc.dram_tensor("wos", [128, 2, D], BF16, kind="ExternalInput").ap()
    out = nc.dram_tensor("out", [T, D], BF16, kind="ExternalOutput").ap()

    with tile.TileContext(nc) as tc:
        with (
            tc.tile_pool(name="persist", bufs=1) as persist,
            tc.tile_pool(name="ptp", bufs=3) as ptp,
            tc.tile_pool(name="normp", bufs=2) as normp,
            tc.tile_pool(name="outp", bufs=3) as outp,
            tc.tile_pool(name="psb", bufs=2, space="PSUM") as psb,  # 2 banks ea
            tc.tile_pool(name="pso", bufs=2, space="PSUM") as pso,  # 1 bank ea
            tc.tile_pool(name="ppp", bufs=2, space="PSUM") as ppp,  # 1 bank ea
        ):
            wq = persist.tile([128, 2, 8, 128], BF16, tag="wq")
            wk = persist.tile([128, 2, 8, 128], BF16, tag="wk")
            wv = persist.tile([128, 8, GC], BF16, tag="wv")
            wo = persist.tile([128, 2, D], BF16, tag="wo")
            xall = persist.tile([128, 4, 8, 512], BF16, tag="xall")
            qt = persist.tile([128, 2, T], BF16, tag="qt")
            kt = persist.tile([128, 2, T], BF16, tag="kt")
            at = persist.tile([128, 2, T], BF16, tag="at")
            vp = persist.tile([128, 16, HPC, DH + 1], BF16, tag="vp")
            ones_sb = persist.tile([128, 64], F32, tag="ones_sb")
            scr = persist.tile([1, 16], F32, tag="scr")

            # ---- input DMAs: first what phase 1 needs first ----
            nc.scalar.dma_start(wq[:, 0, 0:4], wqs[:, 0, 0:4])
            nc.scalar.dma_start(wq[:, 0, 4:8], wqs[:, 0, 4:8])
            nc.scalar.dma_start(wk[:, 0, 0:4], wks[:, 0, 0:4])
            nc.scalar.dma_start(wk[:, 0, 4:8], wks[:, 0, 4:8])
            nc.scalar.dma_start(wv[:], wvs[:])
            nc.scalar.dma_start(wq[:, 1], wqs[:, 1])
            nc.scalar.dma_start(wk[:, 1], wks[:, 1])
            for xq in range(4):
                nc.sync.dma_start(
                    xall[:, 0, 2 * xq : 2 * xq + 2], xs[:, 0, 2 * xq : 2 * xq + 2]
                )
            nc.sync.dma_start(xall[:, 1], xs[:, 1])
            nc.sync.dma_start(xall[:, 2], xs[:, 2])
            nc.sync.dma_start(xall[:, 3], xs[:, 3])
            nc.sync.dma_start(wo[:], wos[:])

            nc.vector.memset(ones_sb[:], 1.0)
            # dummy exp: pulls ACT_TABLE_LOAD into the DMA-wait window
            nc.scalar.activation(scr[0:1, :], ones_sb[0:1, 0:16], EXP, scale=0.125)
            # ones column of V' (row-sum trick)
            nc.vector.tensor_copy(
                vp[:, :, :, DH],
                ones_sb[:].rearrange("p (a b) -> p a b", a=16),
            )
            # bf16 ones block for the K=32 broadcast matmul in the norm chain
            ones_bf = persist.tile([32, 64], BF16, tag="ones_bf")
            nc.vector.tensor_copy(ones_bf[:], ones_sb[0:32, :])

            # dummy matmuls keeping the PE HAM activity window busy while
            # the head of the kernel is DMA-paced (idle PE re-throttles the
            # clock gate to 1.2 GHz for ~3.4us). Uses the score-PSUM ring,
            # which is idle until phase 2 starts.
            warm_tile = psb.tile([128, 2, 512], F32, tag="st", name="warm")

            def warm(n):
                for _ in range(n):
                    nc.tensor.matmul(
                        warm_tile[:, 0, 0:128],
                        wq[:, 0, 0, :],
                        wq[:, 0, 0, :],
                        start=True,
                        stop=True,
                    )

            # ---- phase-1 / phase-3 work units (PE filler granularity) ----
            def unit_qk(tci, which, ct):
                def emit():
                    w_sb, dst = (wq, qt) if which == 0 else (wk, kt)
                    ps = ppp.tile([128, 512], F32, tag="pp", name="ps_qk")
                    for di in range(8):
                        nc.tensor.matmul(
                            ps[:],
                            w_sb[:, ct, di, :],
                            xall[:, tci, di, :],
                            start=(di == 0),
                            stop=(di == 7),
                        )
                        if tci == 0 and ct == 0 and di < 7:
                            warm(1)
                    nc.vector.tensor_copy(
                        dst[:, ct, tci * 512 : (tci + 1) * 512], ps[:]
                    )
                return emit

            def unit_v(tci, tt):
                def emit():
                    ps = ppp.tile([128, GC], F32, tag="pp", name="ps_v")
                    for di in range(8):
                        nc.tensor.matmul(
                            ps[:],
                            xall[:, tci, di, tt * 128 : (tt + 1) * 128],
                            wv[:, di, :],
                            start=(di == 0),
                            stop=(di == 7),
                        )
                        if tci == 0 and tt == 0 and di < 7:
                            warm(1)
                    nc.vector.tensor_copy(
                        vp[:, tci * 4 + tt, :, 0:DH],
                        ps[:].rearrange("p (h d) -> p h d", h=HPC),
                    )
                return emit

            def unit_p3(pc, tt, nn):
                def emit():
                    qti = pc * 4 + tt
                    ps = ppp.tile([128, 512], F32, tag="pp", name="ps_p3")
                    for ctt in range(2):
                        nc.tensor.matmul(
                            ps[:],
                            at[:, ctt, qti * 128 : (qti + 1) * 128],
                            wo[:, ctt, nn * 512 : (nn + 1) * 512],
                            start=(ctt == 0),
                            stop=(ctt == 1),
                        )
                    ot = outp.tile([128, 512], BF16, tag="ot", name="ot")
                    nc.vector.tensor_copy(ot[:], ps[:])
                    nc.sync.dma_start(
                        out[qti * 128 : (qti + 1) * 128, nn * 512 : (nn + 1) * 512],
                        ot[:],
                    )
                return emit

            def p1_units(tci):
                return [
                    unit_qk(tci, 0, 0),
                    unit_qk(tci, 1, 0),
                    unit_v(tci, 0),
                    unit_v(tci, 1),
                    unit_v(tci, 2),
                    unit_v(tci, 3),
                    unit_qk(tci, 0, 1),
                    unit_qk(tci, 1, 1),
                ]

            def p3_units(pc):
                return [unit_p3(pc, tt, nn) for tt in range(4) for nn in range(2)]

            # ---- global deferred-work scheduler (ticks = kti iterations) ----
            tick = [0]
            deferred = []  # sorted-ish list of (due_tick, seq, fn)
            dseq = [0]

            def defer(delay, fn):
                deferred.append((tick[0] + delay, dseq[0], fn))
                dseq[0] += 1

            def run_due():
                deferred.sort(key=lambda x: (x[0], x[1]))
                while deferred and deferred[0][0] <= tick[0]:
                    deferred.pop(0)[2]()

            def drain_deferred():
                deferred.sort(key=lambda x: (x[0], x[1]))
                while deferred:
                    deferred.pop(0)[2]()

            # ---- phase 2 ----
            def phase2(qc, early_fillers, late_fillers, final=False):
                q0 = qc * 512
                n_kt = 4 * (qc + 1)
                iters = [(hp, kti) for hp in range(2) for kti in range(n_kt)]
                I = len(iters)
                emit_at = {}
                for j, f in enumerate(early_fillers):
                    nf = len(early_fillers)
                    slot = min(I - 1, (j * max(1, int(I * 0.7))) // max(1, nf))
                    emit_at.setdefault(slot, []).append(f)
                for j, f in enumerate(late_fillers):
                    nf = len(late_fillers)
                    slot = min(I - 1, int(I * 0.5) + (j * max(1, int(I * 0.45))) // max(1, nf))
                    emit_at.setdefault(slot, []).append(f)

                oo = {}
                pts = {}
                pending_pv = None
                tail_plan = None

                def sc_exp(hp, kti):
                    ct = hp
                    off = max(0, kti * 128 - q0)
                    st = psb.tile([128, 2, 512], F32, tag="st", name="st")
                    for hh in range(2):
                        nc.tensor.matmul(
                            st[:, hh, off:],
                            kt[64 * hh : 64 * hh + 64, ct, kti * 128 : (kti + 1) * 128],
                            qt[64 * hh : 64 * hh + 64, ct, q0 + off : q0 + 512],
                            start=True,
                            stop=True,
                            tile_position=(64 * hh, 0),
                        )
                    pt = ptp.tile([128, 2, 512], BF16, tag="pt", name="pt")
                    nc.scalar.activation(
                        pt[:, :, off:], st[:, :, off:], EXP, scale=0.125
                    )
                    if kti >= 4 * qc:
                        # diagonal: mask the 128-wide band; q' >= p keeps
                        nc.gpsimd.affine_select(
                            out=pt[:, :, off : off + 128],
                            in_=pt[:, :, off : off + 128],
                            compare_op=mybir.AluOpType.is_ge,
                            fill=0.0,
                            base=0,
                            pattern=[[0, 2], [1, 128]],
                            channel_multiplier=-1,
                        )
                    pts[(hp, kti)] = pt

                def emit_pv(hp, kti):
                    off = max(0, kti * 128 - q0)
                    pt = pts.pop((hp, kti))
                    for hh in range(2):
                        nc.tensor.matmul(
                            oo[hp][hh][0 : DH + 1, off:],
                            vp[:, kti, 2 * hp + hh, :],
                            pt[:, hh, off:],
                            start=(kti == 0),
                            stop=(kti == n_kt - 1),
                        )

                def normA(hp, hh, on_scalar=False):
                    def fn():
                        stg = normp.tile([96, 512], BF16, tag=f"stg{hh}", name="stg")
                        cp = nc.scalar.copy if on_scalar else nc.vector.tensor_copy
                        cp(stg[0 : DH + 1, :], oo[hp][hh][0 : DH + 1, :])
                        oo[hp][hh] = None
                        norm_state[(qc, hp, hh)] = stg
                    return fn

                def normB(hp, hh):
                    def fn():
                        stg = norm_state[(qc, hp, hh)]
                        zt = normp.tile([32, 512], BF16, tag=f"zt{hh}", name="zt")
                        nc.vector.transpose(zt[:], stg[64:96, :])
                        zv = zt[:].rearrange("p (a b) -> p a b", b=32)[:, :, 0]
                        with nc.allow_low_precision("bf16 softmax row-sum reciprocal"):
                            nc.vector.reciprocal(zv, zv)
                        norm_state[(qc, hp, hh)] = (stg, zt)
                    return fn

                def normCD(hp, hh, tail=False):
                    def fn():
                        stg, zt = norm_state.pop((qc, hp, hh))
                        zr = normp.tile([32, 512], BF16, tag=f"zr{hh}", name="zr")
                        nc.vector.transpose(zr[:], zt[:])
                        # broadcast 1/Z across 64 partitions with a K=1 matmul;
                        # the multiply follows immediately so the PSUM slot
                        # frees fast. At the tail the filler ring is held by
                        # in-flight out-projection accumulators, so draw from
                        # the freed PV-accumulator ring instead.
                        if tail:
                            rb = pso.tile([64, 512], F32, tag="oo", name="rb")
                        else:
                            rb = ppp.tile([64, 512], F32, tag="pp", name="rb")
                        nc.tensor.matmul(
                            rb[:], ones_bf[0:1, :], zr[0:1, :], start=True, stop=True
                        )
                        nc.vector.tensor_mul(
                            at[64 * hh : 64 * hh + 64, hp, q0 : q0 + 512],
                            stg[0:DH, :],
                            rb[:],
                        )
                    return fn

                for i, (hp, kti) in enumerate(iters):
                    if kti == 0:
                        oo[hp] = [
                            pso.tile([DH + 1, 512], F32, tag="oo", name="oo0"),
                            pso.tile([DH + 1, 512], F32, tag="oo", name="oo1"),
                        ]
                    sc_exp(hp, kti)
                    if pending_pv is not None:
                        emit_pv(*pending_pv)
                    pending_pv = (hp, kti)
                    run_due()
                    for f in emit_at.get(i, []):
                        f()
                    if kti == n_kt - 1:
                        emit_pv(*pending_pv)
                        pending_pv = None
                        if final and hp == 1:
                            tail_plan = {
                                "A0": normA(1, 0, on_scalar=True),
                                "A1": normA(1, 1, on_scalar=True),
                                "B0": normB(1, 0),
                                "B1": normB(1, 1),
                                "CD0": normCD(1, 0, tail=True),
                                "CD1": normCD(1, 1, tail=True),
                            }
                        else:
                            for hh in range(2):
                                defer(1 + hh, normA(hp, hh))
                                defer(2 + hh, normB(hp, hh))
                                defer(3 + hh, normCD(hp, hh))
                    tick[0] += 1
                return tail_plan

            norm_state = {}

            # ---- driver ----
            for u in p1_units(0):
                u()
            phase2(0, p1_units(1), [])
            phase2(1, p1_units(2), [])
            phase2(2, p1_units(3), p3_units(0))
            tail = phase2(3, [], p3_units(1) + p3_units(2), final=True)
            drain_deferred()
            # ---- hand-scheduled tail: final head-pair normalization
            # interleaved with the last out-projection blocks ----
            tail["A0"]()
            tail["A1"]()
            tail["B0"]()
            tail["B1"]()
            # qti=12/13 ct0 matmuls run while the reciprocal pipeline drains
            # (qti 13 borrows the idle score-PSUM ring)
            ps_pre = []
            for tt in range(2):
                blk = slice((12 + tt) * 128, (13 + tt) * 128)
                for nn in range(2):
                    pool, tag = (ppp, "pp") if tt == 0 else (psb, "st")
                    ps = pool.tile([128, 512], F32, tag=tag, name="ps_pre")
                    nc.tensor.matmul(
                        ps[:],
                        at[:, 0, blk],
                        wo[:, 0, nn * 512 : (nn + 1) * 512],
                        start=True,
                        stop=False,
                    )
                    ps_pre.append((blk, nn, ps))
            tail["CD0"]()
            tail["CD1"]()
            for blk, nn, ps in ps_pre:
                nc.tensor.matmul(
                    ps[:],
                    at[:, 1, blk],
                    wo[:, 1, nn * 512 : (nn + 1) * 512],
                    start=False,
                    stop=True,
                )
                ot = outp.tile([128, 512], BF16, tag="ot", name="ot12")
                nc.vector.tensor_copy(ot[:], ps[:])
                nc.sync.dma_start(out[blk, nn * 512 : (nn + 1) * 512], ot[:])
            for tt in range(2, 4):
                unit_p3(3, tt, 0)()
                unit_p3(3, tt, 1)()
    nc.compile()
    return nc


def _get_nc():
    if "nc" not in _CACHE:
        _CACHE["nc"] = _build()
    return _CACHE["nc"]


def _in_maps(x, Wq, Wk, Wv, Wo):
    bf16 = ml_dtypes.bfloat16
    x = np.asarray(x, dtype=np.float32)
    xb = [
        np.ascontiguousarray(
            x[b].reshape(4, 512, 8, 128).transpose(3, 0, 2, 1)
        ).astype(bf16)
        for b in range(B)
    ]
    Wq = np.asarray(Wq, dtype=np.float32).astype(bf16)
    Wk = np.asarray(Wk, dtype=np.float32).astype(bf16)
    Wv = np.asarray(Wv, dtype=np.float32).astype(bf16)
    Wo = np.asarray(Wo, dtype=np.float32).astype(bf16)
    maps = []
    for core in range(N_CORES):
        b, g = divmod(core, 4)
        sl = slice(g * GC, (g + 1) * GC)
        # wq/wk[p, ct, dt, c] = W[sl][ct*128+c, dt*128+p]
        wqw = np.ascontiguousarray(
            Wq[sl].reshape(2, 128, 8, 128).transpose(3, 0, 2, 1)
        )
        wkw = np.ascontiguousarray(
            Wk[sl].reshape(2, 128, 8, 128).transpose(3, 0, 2, 1)
        )
        # wv[p, dt, c] = Wv[sl][c, dt*128+p]
        wvw = np.ascontiguousarray(Wv[sl].reshape(GC, 8, 128).transpose(2, 1, 0))
        # wo[p, ct, n] = Wo[n, g*256 + ct*128 + p]
        wow = np.ascontiguousarray(Wo[:, sl].reshape(D, 2, 128).transpose(2, 1, 0))
        maps.append(
            {
                "xs": xb[b],
                "wqs": wqw,
                "wks": wkw,
                "wvs": wvw,
                "wos": wow,
            }
        )
    return maps


def _run(x, Wq, Wk, Wv, Wo, **spmd_kwargs):
    nc = _get_nc()
    res = run_bass_kernel_spmd(
        nc, _in_maps(x, Wq, Wk, Wv, Wo), core_ids=list(range(N_CORES)), **spmd_kwargs
    )
    outs = [np.asarray(r["out"], dtype=np.float32) for r in res.results]
    full = np.stack(
        [
            outs[0] + outs[1] + outs[2] + outs[3],
            outs[4] + outs[5] + outs[6] + outs[7],
        ]
    ).astype(np.float32)
    return full, res


def kernel(x, Wq, Wk, Wv, Wo):
    full, _ = _run(x, Wq, Wk, Wv, Wo)
    return full


# revision 35
# speedup vs baseline: 1.0024x; 1.0024x over previous
"""Causal multi-head self-attention (B=2, T=2048, D=1024, H=16) on 8 TRN2
NeuronCores.

Sharding (Megatron-style, hardcoded): core = 4*b + g where b in {0,1} is the
batch and g in {0..3} a group of 4 heads. Each core computes Q/K/V projections
for its head group from x[b], fused causal attention for those 4 heads, and a
partial output projection against its 256-column slice of Wo. The host sums
the 4 partial outputs per batch (the all-reduce after out_proj).

v3 design notes:
 - All matmul operands bf16 (PSUM accumulation f32): halves input DMA bytes,
   keeps matmuls at 1 row/cycle at any free size, enabling exact-causal
   N-slicing of diagonal score/PV tiles.
 - Input DMAs sized >=256KB (ct-major Wq/Wk layout) split across the scalar
   and sync HWDGE queues so the first projection matmul starts ~7us in.
 - Scores transposed (S^T[k, q]) per head pair on disjoint PE row groups
   (concurrent when warm); exp on Scalar writes bf16; diagonal 128-col band
   masked post-exp with gpsimd.affine_select.
 - Row sums via a ones-column appended to V (row 64 of the PV accumulator).
   Normalization is DMA-free: Z row reshaped via DVE 32x32 stream transpose,
   multi-lane reciprocal on the strided view, transpose back, gpsimd
   partition_broadcast, divide folded into the bf16 `at` write.
 - Software pipelining: PV(kti) emitted after scores(kti+1); Q/K/V units of
   chunk qc+1 and out-projection units of earlier chunks interleave into the
   kti loop as PE filler (keeps HAM at K=8/8). Norm stages run as deferred
   work spread over subsequent iterations (crossing chunk boundaries) so no
   engine FIFO blocks on them.
"""

from collections import deque

import numpy as np
import ml_dtypes

import concourse.bass as bass
import concourse.tile as tile
from concourse import bacc, mybir
from concourse.bass_utils import run_bass_kernel_spmd

B, T, D, H, DH = 2, 2048, 1024, 16, 64
HPC = 4  # heads per core
GC = 256  # projection columns per core (HPC * DH)
N_CORES = 8
F32 = mybir.dt.float32
BF16 = mybir.dt.bfloat16
EXP = mybir.ActivationFunctionType.Exp

_CACHE = {}


def _build():
    nc = bacc.Bacc(
        "TRN2", target_bir_lowering=False, debug=False, num_devices=N_CORES
    )
    # Pre-swizzled inputs (host does the transposes + bf16 cast):
    #   xs[p, tc, dt, t] = x[b, tc*512+t, dt*128+p]
    #   wq/wk[p, ct, dt, c] = W[g*256 + ct*128 + c, dt*128+p]
    #   wv[p, dt, c] = Wv[g*256+c, dt*128+p]
    #   wo[p, ct, n] = Wo[n, g*256 + ct*128 + p]
    xs = nc.dram_tensor("xs", [128, 4, 8, 512], BF16, kind="ExternalInput").ap()
    wqs = nc.dram_tensor("wqs", [128, 2, 8, 128], BF16, kind="ExternalInput").ap()
    wks = nc.dram_tensor("wks", [128, 2, 8, 128], BF16, kind="ExternalInput").ap()
    wvs = nc.dram_tensor("wvs", [128, 8, GC], BF16, kind="ExternalInput").ap()
    wos = nc.dram_tensor("wos", [128, 2, D], BF16, kind="ExternalInput").ap()
    out = nc.dram_tensor("out", [T, D], BF16, kind="ExternalOutput").ap()

    with tile.TileContext(nc) as tc:
        with (
            tc.tile_pool(name="persist", bufs=1) as persist,
            tc.tile_pool(name="ptp", bufs=3) as ptp,
            tc.tile_pool(name="normp", bufs=2) as normp,
            tc.tile_pool(name="outp", bufs=3) as outp,
            tc.tile_pool(name="psb", bufs=2, space="PSUM") as psb,  # 2 banks ea
            tc.tile_pool(name="pso", bufs=2, space="PSUM") as pso,  # 1 bank ea
            tc.tile_pool(name="ppp", bufs=2, space="PSUM") as ppp,  # 1 bank ea
        ):
            wq = persist.tile([128, 2, 8, 128], BF16, tag="wq")
            wk = persist.tile([128, 2, 8, 128], BF16, tag="wk")
            wv = persist.tile([128, 8, GC], BF16, tag="wv")
            wo = persist.tile([128, 2, D], BF16, tag="wo")
            xall = persist.tile([128, 4, 8, 512], BF16, tag="xall")
            qt = persist.tile([128, 2, T], BF16, tag="qt")
            kt = persist.tile([128, 2, T], BF16, tag="kt")
            at = persist.tile([128, 2, T], BF16, tag="at")
            vp = persist.tile([128, 16, HPC, DH + 1], BF16, tag="vp")
            ones_sb = persist.tile([128, 64], F32, tag="ones_sb")
            scr = persist.tile([1, 16], F32, tag="scr")

            # ---- input DMAs: first what phase 1 needs first ----
            nc.scalar.dma_start(wq[:, 0, 0:4], wqs[:, 0, 0:4])
            nc.scalar.dma_start(wq[:, 0, 4:8], wqs[:, 0, 4:8])
            nc.scalar.dma_start(wk[:, 0, 0:4], wks[:, 0, 0:4])
            nc.scalar.dma_start(wk[:, 0, 4:8], wks[:, 0, 4:8])
            nc.scalar.dma_start(wv[:], wvs[:])
            nc.scalar.dma_start(wq[:, 1], wqs[:, 1])
            nc.scalar.dma_start(wk[:, 1], wks[:, 1])
            for xq in range(4):
                nc.sync.dma_start(
                    xall[:, 0, 2 * xq : 2 * xq + 2], xs[:, 0, 2 * xq : 2 * xq + 2]
                )
            nc.sync.dma_start(xall[:, 1], xs[:, 1])
            nc.sync.dma_start(xall[:, 2], xs[:, 2])
            nc.sync.dma_start(xall[:, 3], xs[:, 3])
            nc.sync.dma_start(wo[:], wos[:])

            nc.vector.memset(ones_sb[:], 1.0)
            # dummy exp: pulls ACT_TABLE_LOAD into the DMA-wait window
            nc.scalar.activation(scr[0:1, :], ones_sb[0:1, 0:16], EXP, scale=0.125)
            # ones column of V' (row-sum trick)
            nc.vector.tensor_copy(
                vp[:, :, :, DH],
                ones_sb[:].rearrange("p (a b) -> p a b", a=16),
            )
            # bf16 ones block for the K=32 broadcast matmul in the norm chain
            ones_bf = persist.tile([32, 64], BF16, tag="ones_bf")
            nc.vector.tensor_copy(ones_bf[:], ones_sb[0:32, :])

            # dummy matmuls keeping the PE HAM activity window busy while
            # the head of the kernel is DMA-paced (idle PE re-throttles the
            # clock gate to 1.2 GHz for ~3.4us). Uses the score-PSUM ring,
            # which is idle until phase 2 starts.
            warm_tile = psb.tile([128, 2, 512], F32, tag="st", name="warm")

            def warm(n):
                for _ in range(n):
                    nc.tensor.matmul(
                        warm_tile[:, 0, 0:128],
                        wq[:, 0, 0, :],
                        wq[:, 0, 0, :],
                        start=True,
                        stop=True,
                    )

            # ---- phase-1 / phase-3 work units (PE filler granularity) ----
            def unit_qk(tci, which, ct):
                def emit():
                    w_sb, dst = (wq, qt) if which == 0 else (wk, kt)
                    ps = ppp.tile([128, 512], F32, tag="pp", name="ps_qk")
                    for di in range(8):
                        nc.tensor.matmul(
                            ps[:],
                            w_sb[:, ct, di, :],
                            xall[:, tci, di, :],
                            start=(di == 0),
                            stop=(di == 7),
                        )
                        if tci == 0 and ct == 0 and di < 7:
                            warm(1)
                    nc.vector.tensor_copy(
                        dst[:, ct, tci * 512 : (tci + 1) * 512], ps[:]
                    )
                return emit

            def unit_v(tci, tt):
                def emit():
                    ps = ppp.tile([128, GC], F32, tag="pp", name="ps_v")
                    for di in range(8):
                        nc.tensor.matmul(
                            ps[:],
                            xall[:, tci, di, tt * 128 : (tt + 1) * 128],
                            wv[:, di, :],
                            start=(di == 0),
                            stop=(di == 7),
                        )
                        if tci == 0 and tt == 0 and di < 7:
                            warm(1)
                    nc.vector.tensor_copy(
                        vp[:, tci * 4 + tt, :, 0:DH],
                        ps[:].rearrange("p (h d) -> p h d", h=HPC),
                    )
                return emit

            def unit_p3(pc, tt, nn):
                def emit():
                    qti = pc * 4 + tt
                    ps = ppp.tile([128, 512], F32, tag="pp", name="ps_p3")
                    for ctt in range(2):
                        nc.tensor.matmul(
                            ps[:],
                            at[:, ctt, qti * 128 : (qti + 1) * 128],
                            wo[:, ctt, nn * 512 : (nn + 1) * 512],
                            start=(ctt == 0),
                            stop=(ctt == 1),
                        )
                    ot = outp.tile([128, 512], BF16, tag="ot", name="ot")
                    nc.vector.tensor_copy(ot[:], ps[:])
                    nc.sync.dma_start(
                        out[qti * 128 : (qti + 1) * 128, nn * 512 : (nn + 1) * 512],
                        ot[:],
                    )
                return emit

            def p1_units(tci):
                return [
                    unit_qk(tci, 0, 0),
                    unit_qk(tci, 1, 0),
                    unit_v(tci, 0),
                    unit_v(tci, 1),
                    unit_v(tci, 2),
                    unit_v(tci, 3),
                    unit_qk(tci, 0, 1),
                    unit_qk(tci, 1, 1),
                ]

            def p3_units(pc):
                return [unit_p3(pc, tt, nn) for tt in range(4) for nn in range(2)]

            # ---- global deferred-work scheduler (ticks = kti iterations) ----
            tick = [0]
            deferred = []  # sorted-ish list of (due_tick, seq, fn)
            dseq = [0]

            def defer(delay, fn):
                deferred.append((tick[0] + delay, dseq[0], fn))
                dseq[0] += 1

            def run_due():
                deferred.sort(key=lambda x: (x[0], x[1]))
                while deferred and deferred[0][0] <= tick[0]:
                    deferred.pop(0)[2]()

            def drain_deferred():
                deferred.sort(key=lambda x: (x[0], x[1]))
                while deferred:
                    deferred.pop(0)[2]()

            # ---- phase 2 ----
            def phase2(qc, early_fillers, late_fillers, final=False):
                q0 = qc * 512
                n_kt = 4 * (qc + 1)
                iters = [(hp, kti) for hp in range(2) for kti in range(n_kt)]
                I = len(iters)
                emit_at = {}
                for j, f in enumerate(early_fillers):
                    nf = len(early_fillers)
                    slot = min(I - 1, (j * max(1, int(I * 0.7))) // max(1, nf))
                    emit_at.setdefault(slot, []).append(f)
                for j, f in enumerate(late_fillers):
                    nf = len(late_fillers)
                    slot = min(I - 1, int(I * 0.5) + (j * max(1, int(I * 0.45))) // max(1, nf))
                    emit_at.setdefault(slot, []).append(f)

                oo = {}
                pts = {}
                pending_pv = None
                tail_plan = None

                def sc_exp(hp, kti):
                    ct = hp
                    off = max(0, kti * 128 - q0)
                    st = psb.tile([128, 2, 512], F32, tag="st", name="st")
                    for hh in range(2):
                        nc.tensor.matmul(
                            st[:, hh, off:],
                            kt[64 * hh : 64 * hh + 64, ct, kti * 128 : (kti + 1) * 128],
                            qt[64 * hh : 64 * hh + 64, ct, q0 + off : q0 + 512],
                            start=True,
                            stop=True,
                            tile_position=(64 * hh, 0),
                        )
                    pt = ptp.tile([128, 2, 512], BF16, tag="pt", name="pt")
                    nc.scalar.activation(
                        pt[:, :, off:], st[:, :, off:], EXP, scale=0.125
                    )
                    if kti >= 4 * qc:
                        # diagonal: mask the 128-wide band; q' >= p keeps
                        nc.gpsimd.affine_select(
                            out=pt[:, :, off : off + 128],
                            in_=pt[:, :, off : off + 128],
                            compare_op=mybir.AluOpType.is_ge,
                            fill=0.0,
                            base=0,
                            pattern=[[0, 2], [1, 128]],
                            channel_multiplier=-1,
                        )
                    pts[(hp, kti)] = pt

                def emit_pv(hp, kti):
                    off = max(0, kti * 128 - q0)
                    pt = pts.pop((hp, kti))
                    for hh in range(2):
                        nc.tensor.matmul(
                            oo[hp][hh][0 : DH + 1, off:],
                            vp[:, kti, 2 * hp + hh, :],
                            pt[:, hh, off:],
                            start=(kti == 0),
                            stop=(kti == n_kt - 1),
                        )

                def normA(hp, hh, on_scalar=False):
                    def fn():
                        stg = normp.tile([96, 512], BF16, tag=f"stg{hh}", name="stg")
                        cp = nc.scalar.copy if on_scalar else nc.vector.tensor_copy
                        cp(stg[0 : DH + 1, :], oo[hp][hh][0 : DH + 1, :])
                        oo[hp][hh] = None
                        norm_state[(qc, hp, hh)] = stg
                    return fn

                def normB(hp, hh):
                    def fn():
                        stg = norm_state[(qc, hp, hh)]
                        zt = normp.tile([32, 512], BF16, tag=f"zt{hh}", name="zt")
                        nc.vector.transpose(zt[:], stg[64:96, :])
                        zv = zt[:].rearrange("p (a b) -> p a b", b=32)[:, :, 0]
                        with nc.allow_low_precision("bf16 softmax row-sum reciprocal"):
                            nc.vector.reciprocal(zv, zv)
                        zr = normp.tile([32, 512], BF16, tag=f"zr{hh}", name="zr")
                        nc.vector.transpose(zr[:], zt[:])
                        norm_state[(qc, hp, hh)] = (stg, zr)
                    return fn

                def normCD(hp, hh, tail=False):
                    def fn():
                        stg, zr = norm_state.pop((qc, hp, hh))
                        # broadcast 1/Z across 64 partitions with a K=1 matmul;
                        # the multiply follows immediately so the PSUM slot
                        # frees fast. At the tail the filler ring is held by
                        # in-flight out-projection accumulators, so draw from
                        # the freed PV-accumulator ring instead.
                        if tail:
                            rb = pso.tile([64, 512], F32, tag="oo", name="rb")
                        else:
                            rb = ppp.tile([64, 512], F32, tag="pp", name="rb")
                        nc.tensor.matmul(
                            rb[:], ones_bf[0:1, :], zr[0:1, :], start=True, stop=True
                        )
                        nc.vector.tensor_mul(
                            at[64 * hh : 64 * hh + 64, hp, q0 : q0 + 512],
                            stg[0:DH, :],
                            rb[:],
                        )
                    return fn

                for i, (hp, kti) in enumerate(iters):
                    if kti == 0:
                        oo[hp] = [
                            pso.tile([DH + 1, 512], F32, tag="oo", name="oo0"),
                            pso.tile([DH + 1, 512], F32, tag="oo", name="oo1"),
                        ]
                    sc_exp(hp, kti)
                    run_due()
                    for f in emit_at.get(i, []):
                        f()
                    if pending_pv is not None:
                        emit_pv(*pending_pv)
                    pending_pv = (hp, kti)
                    if kti == n_kt - 1:
                        emit_pv(*pending_pv)
                        pending_pv = None
                        if final and hp == 1:
                            tail_plan = {
                                "A0": normA(1, 0, on_scalar=True),
                                "A1": normA(1, 1, on_scalar=True),
                                "B0": normB(1, 0),
                                "B1": normB(1, 1),
                                "CD0": normCD(1, 0, tail=True),
                                "CD1": normCD(1, 1, tail=True),
                            }
                        else:
                            for hh in range(2):
                                defer(1 + hh, normA(hp, hh))
                                defer(2 + hh, normB(hp, hh))
                                defer(3 + hh, normCD(hp, hh))
                    tick[0] += 1
                return tail_plan

            norm_state = {}

            # ---- driver ----
            for u in p1_units(0):
                u()
            phase2(0, p1_units(1), [])
            phase2(1, p1_units(2), [])
            phase2(2, p1_units(3), p3_units(0))
            tail = phase2(3, [], p3_units(1) + p3_units(2), final=True)
            drain_deferred()
            # ---- hand-scheduled tail: final head-pair normalization
            # interleaved with the last out-projection blocks ----
            tail["A0"]()
            tail["A1"]()
            tail["B0"]()
            tail["B1"]()
            # qti=12/13 ct0 matmuls run while the reciprocal pipeline drains
            # (qti 13 borrows the idle score-PSUM ring)
            ps_pre = []
            for tt in range(2):
                blk = slice((12 + tt) * 128, (13 + tt) * 128)
                for nn in range(2):
                    pool, tag = (ppp, "pp") if tt == 0 else (psb, "st")
                    ps = pool.tile([128, 512], F32, tag=tag, name="ps_pre")
                    nc.tensor.matmul(
                        ps[:],
                        at[:, 0, blk],
                        wo[:, 0, nn * 512 : (nn + 1) * 512],
                        start=True,
                        stop=False,
                    )
                    ps_pre.append((blk, nn, ps))
            tail["CD0"]()
            tail["CD1"]()
            for blk, nn, ps in ps_pre:
                nc.tensor.matmul(
                    ps[:],
                    at[:, 1, blk],
                    wo[:, 1, nn * 512 : (nn + 1) * 512],
                    start=False,
                    stop=True,
                )
                ot = outp.tile([128, 512], BF16, tag="ot", name="ot12")
                nc.vector.tensor_copy(ot[:], ps[:])
                nc.sync.dma_start(out[blk, nn * 512 : (nn + 1) * 512], ot[:])
            for tt in range(2, 4):
                unit_p3(3, tt, 0)()
                unit_p3(3, tt, 1)()
    nc.compile()
    return nc


def _get_nc():
    if "nc" not in _CACHE:
        _CACHE["nc"] = _build()
    return _CACHE["nc"]


def _in_maps(x, Wq, Wk, Wv, Wo):
    bf16 = ml_dtypes.bfloat16
    x = np.asarray(x, dtype=np.float32)
    xb = [
        np.ascontiguousarray(
            x[b].reshape(4, 512, 8, 128).transpose(3, 0, 2, 1)
        ).astype(bf16)
        for b in range(B)
    ]
    Wq = np.asarray(Wq, dtype=np.float32).astype(bf16)
    Wk = np.asarray(Wk, dtype=np.float32).astype(bf16)
    Wv = np.asarray(Wv, dtype=np.float32).astype(bf16)
    Wo = np.asarray(Wo, dtype=np.float32).astype(bf16)
    maps = []
    for core in range(N_CORES):
        b, g = divmod(core, 4)
        sl = slice(g * GC, (g + 1) * GC)
        # wq/wk[p, ct, dt, c] = W[sl][ct*128+c, dt*128+p]
        wqw = np.ascontiguousarray(
            Wq[sl].reshape(2, 128, 8, 128).transpose(3, 0, 2, 1)
        )
        wkw = np.ascontiguousarray(
            Wk[sl].reshape(2, 128, 8, 128).transpose(3, 0, 2, 1)
        )
        # wv[p, dt, c] = Wv[sl][c, dt*128+p]
        wvw = np.ascontiguousarray(Wv[sl].reshape(GC, 8, 128).transpose(2, 1, 0))
        # wo[p, ct, n] = Wo[n, g*256 + ct*128 + p]
        wow = np.ascontiguousarray(Wo[:, sl].reshape(D, 2, 128).transpose(2, 1, 0))
        maps.append(
            {
                "xs": xb[b],
                "wqs": wqw,
                "wks": wkw,
                "wvs": wvw,
                "wos": wow,
            }
        )
    return maps


def _run(x, Wq, Wk, Wv, Wo, **spmd_kwargs):
    nc = _get_nc()
    res = run_bass_kernel_spmd(
        nc, _in_maps(x, Wq, Wk, Wv, Wo), core_ids=list(range(N_CORES)), **spmd_kwargs
    )
    outs = [np.asarray(r["out"], dtype=np.float32) for r in res.results]
    full = np.stack(
        [
            outs[0] + outs[1] + outs[2] + outs[3],
            outs[4] + outs[5] + outs[6] + outs[7],
        ]
    ).astype(np.float32)
    return full, res


def kernel(x, Wq, Wk, Wv, Wo):
    full, _ = _run(x, Wq, Wk, Wv, Wo)
    return full


# revision 36
# speedup vs baseline: 1.0165x; 1.0140x over previous
"""Causal multi-head self-attention (B=2, T=2048, D=1024, H=16) on 8 TRN2
NeuronCores.

Sharding (Megatron-style, hardcoded): core = 4*b + g where b in {0,1} is the
batch and g in {0..3} a group of 4 heads. Each core computes Q/K/V projections
for its head group from x[b], fused causal attention for those 4 heads, and a
partial output projection against its 256-column slice of Wo. The host sums
the 4 partial outputs per batch (the all-reduce after out_proj).

v3 design notes:
 - All matmul operands bf16 (PSUM accumulation f32): halves input DMA bytes,
   keeps matmuls at 1 row/cycle at any free size, enabling exact-causal
   N-slicing of diagonal score/PV tiles.
 - Input DMAs sized >=256KB (ct-major Wq/Wk layout) split across the scalar
   and sync HWDGE queues so the first projection matmul starts ~7us in.
 - Scores transposed (S^T[k, q]) per head pair on disjoint PE row groups
   (concurrent when warm); exp on Scalar writes bf16; diagonal 128-col band
   masked post-exp with gpsimd.affine_select.
 - Row sums via a ones-column appended to V (row 64 of the PV accumulator).
   Normalization is DMA-free: Z row reshaped via DVE 32x32 stream transpose,
   multi-lane reciprocal on the strided view, transpose back, gpsimd
   partition_broadcast, divide folded into the bf16 `at` write.
 - Software pipelining: PV(kti) emitted after scores(kti+1); Q/K/V units of
   chunk qc+1 and out-projection units of earlier chunks interleave into the
   kti loop as PE filler (keeps HAM at K=8/8). Norm stages run as deferred
   work spread over subsequent iterations (crossing chunk boundaries) so no
   engine FIFO blocks on them.
"""

from collections import deque

import numpy as np
import ml_dtypes

import concourse.bass as bass
import concourse.tile as tile
from concourse import bacc, mybir
from concourse.bass_utils import run_bass_kernel_spmd

B, T, D, H, DH = 2, 2048, 1024, 16, 64
HPC = 4  # heads per core
GC = 256  # projection columns per core (HPC * DH)
N_CORES = 8
F32 = mybir.dt.float32
BF16 = mybir.dt.bfloat16
EXP = mybir.ActivationFunctionType.Exp

_CACHE = {}


def _build():
    nc = bacc.Bacc(
        "TRN2", target_bir_lowering=False, debug=False, num_devices=N_CORES
    )
    # Pre-swizzled inputs (host does the transposes + bf16 cast):
    #   xs[p, tc, dt, t] = x[b, tc*512+t, dt*128+p]
    #   wq/wk[p, ct, dt, c] = W[g*256 + ct*128 + c, dt*128+p]
    #   wv[p, dt, c] = Wv[g*256+c, dt*128+p]
    #   wo[p, ct, n] = Wo[n, g*256 + ct*128 + p]
    xs = nc.dram_tensor("xs", [128, 4, 8, 512], BF16, kind="ExternalInput").ap()
    wqs = nc.dram_tensor("wqs", [128, 2, 8, 128], BF16, kind="ExternalInput").ap()
    wks = nc.dram_tensor("wks", [128, 2, 8, 128], BF16, kind="ExternalInput").ap()
    wvs = nc.dram_tensor("wvs", [128, 8, GC], BF16, kind="ExternalInput").ap()
    wos = nc.dram_tensor("wos", [128, 2, D], BF16, kind="ExternalInput").ap()
    out = nc.dram_tensor("out", [T, D], BF16, kind="ExternalOutput").ap()

    with tile.TileContext(nc) as tc:
        with (
            tc.tile_pool(name="persist", bufs=1) as persist,
            tc.tile_pool(name="ptp", bufs=3) as ptp,
            tc.tile_pool(name="normp", bufs=2) as normp,
            tc.tile_pool(name="outp", bufs=3) as outp,
            tc.tile_pool(name="psb", bufs=2, space="PSUM") as psb,  # 2 banks ea
            tc.tile_pool(name="pso", bufs=2, space="PSUM") as pso,  # 1 bank ea
            tc.tile_pool(name="ppp", bufs=2, space="PSUM") as ppp,  # 1 bank ea
        ):
            wq = persist.tile([128, 2, 8, 128], BF16, tag="wq")
            wk = persist.tile([128, 2, 8, 128], BF16, tag="wk")
            wv = persist.tile([128, 8, GC], BF16, tag="wv")
            wo = persist.tile([128, 2, D], BF16, tag="wo")
            xall = persist.tile([128, 4, 8, 512], BF16, tag="xall")
            qt = persist.tile([128, 2, T], BF16, tag="qt")
            kt = persist.tile([128, 2, T], BF16, tag="kt")
            at = persist.tile([128, 2, T], BF16, tag="at")
            vp = persist.tile([128, 16, HPC, DH + 1], BF16, tag="vp")
            ones_sb = persist.tile([128, 64], F32, tag="ones_sb")
            scr = persist.tile([1, 16], F32, tag="scr")

            # ---- input DMAs: first what phase 1 needs first ----
            nc.scalar.dma_start(wq[:, 0, 0:4], wqs[:, 0, 0:4])
            nc.scalar.dma_start(wq[:, 0, 4:8], wqs[:, 0, 4:8])
            nc.scalar.dma_start(wk[:, 0, 0:4], wks[:, 0, 0:4])
            nc.scalar.dma_start(wk[:, 0, 4:8], wks[:, 0, 4:8])
            nc.scalar.dma_start(wv[:], wvs[:])
            nc.scalar.dma_start(wq[:, 1], wqs[:, 1])
            nc.scalar.dma_start(wk[:, 1], wks[:, 1])
            for xq in range(4):
                nc.sync.dma_start(
                    xall[:, 0, 2 * xq : 2 * xq + 2], xs[:, 0, 2 * xq : 2 * xq + 2]
                )
            nc.sync.dma_start(xall[:, 1], xs[:, 1])
            nc.sync.dma_start(xall[:, 2], xs[:, 2])
            nc.sync.dma_start(xall[:, 3], xs[:, 3])
            nc.sync.dma_start(wo[:], wos[:])

            nc.vector.memset(ones_sb[:], 1.0)
            # dummy exp: pulls ACT_TABLE_LOAD into the DMA-wait window
            nc.scalar.activation(scr[0:1, :], ones_sb[0:1, 0:16], EXP, scale=0.125)
            # ones column of V' (row-sum trick)
            nc.vector.tensor_copy(
                vp[:, :, :, DH],
                ones_sb[:].rearrange("p (a b) -> p a b", a=16),
            )
            # bf16 ones block for the K=32 broadcast matmul in the norm chain
            ones_bf = persist.tile([32, 64], BF16, tag="ones_bf")
            nc.vector.tensor_copy(ones_bf[:], ones_sb[0:32, :])

            # dummy matmuls keeping the PE HAM activity window busy while
            # the head of the kernel is DMA-paced (idle PE re-throttles the
            # clock gate to 1.2 GHz for ~3.4us). Uses the score-PSUM ring,
            # which is idle until phase 2 starts.
            warm_tile = psb.tile([128, 2, 512], F32, tag="st", name="warm")

            def warm(n):
                for _ in range(n):
                    nc.tensor.matmul(
                        warm_tile[:, 0, 0:128],
                        wq[:, 0, 0, :],
                        wq[:, 0, 0, :],
                        start=True,
                        stop=True,
                    )

            # ---- phase-1 / phase-3 work units (PE filler granularity) ----
            def unit_qk(tci, which, ct):
                def emit():
                    w_sb, dst = (wq, qt) if which == 0 else (wk, kt)
                    ps = ppp.tile([128, 512], F32, tag="pp", name="ps_qk")
                    for di in range(8):
                        nc.tensor.matmul(
                            ps[:],
                            w_sb[:, ct, di, :],
                            xall[:, tci, di, :],
                            start=(di == 0),
                            stop=(di == 7),
                        )
                        if tci == 0 and ct == 0 and di < 7:
                            warm(1)
                    nc.vector.tensor_copy(
                        dst[:, ct, tci * 512 : (tci + 1) * 512], ps[:]
                    )
                return emit

            def unit_v(tci, tt):
                def emit():
                    ps = ppp.tile([128, GC], F32, tag="pp", name="ps_v")
                    for di in range(8):
                        nc.tensor.matmul(
                            ps[:],
                            xall[:, tci, di, tt * 128 : (tt + 1) * 128],
                            wv[:, di, :],
                            start=(di == 0),
                            stop=(di == 7),
                        )
                        if tci == 0 and tt == 0 and di < 7:
                            warm(1)
                    nc.vector.tensor_copy(
                        vp[:, tci * 4 + tt, :, 0:DH],
                        ps[:].rearrange("p (h d) -> p h d", h=HPC),
                    )
                return emit

            def unit_p3(pc, tt, nn):
                def emit():
                    qti = pc * 4 + tt
                    ps = ppp.tile([128, 512], F32, tag="pp", name="ps_p3")
                    for ctt in range(2):
                        nc.tensor.matmul(
                            ps[:],
                            at[:, ctt, qti * 128 : (qti + 1) * 128],
                            wo[:, ctt, nn * 512 : (nn + 1) * 512],
                            start=(ctt == 0),
                            stop=(ctt == 1),
                        )
                    ot = outp.tile([128, 512], BF16, tag="ot", name="ot")
                    nc.vector.tensor_copy(ot[:], ps[:])
                    nc.sync.dma_start(
                        out[qti * 128 : (qti + 1) * 128, nn * 512 : (nn + 1) * 512],
                        ot[:],
                    )
                return emit

            def p1_units(tci):
                return [
                    unit_qk(tci, 0, 0),
                    unit_qk(tci, 1, 0),
                    unit_v(tci, 0),
                    unit_v(tci, 1),
                    unit_v(tci, 2),
                    unit_v(tci, 3),
                    unit_qk(tci, 0, 1),
                    unit_qk(tci, 1, 1),
                ]

            def p3_units(pc):
                return [unit_p3(pc, tt, nn) for tt in range(4) for nn in range(2)]

            # ---- global deferred-work scheduler (ticks = kti iterations) ----
            tick = [0]
            deferred = []  # sorted-ish list of (due_tick, seq, fn)
            dseq = [0]

            def defer(delay, fn):
                deferred.append((tick[0] + delay, dseq[0], fn))
                dseq[0] += 1

            def run_due():
                deferred.sort(key=lambda x: (x[0], x[1]))
                while deferred and deferred[0][0] <= tick[0]:
                    deferred.pop(0)[2]()

            def drain_deferred():
                deferred.sort(key=lambda x: (x[0], x[1]))
                while deferred:
                    deferred.pop(0)[2]()

            # ---- phase 2 ----
            def phase2(qc, early_fillers, late_fillers, final=False):
                q0 = qc * 512
                n_kt = 4 * (qc + 1)
                iters = [(hp, kti) for hp in range(2) for kti in range(n_kt)]
                I = len(iters)
                emit_at = {}
                for j, f in enumerate(early_fillers):
                    nf = len(early_fillers)
                    slot = min(I - 1, (j * max(1, int(I * 0.7))) // max(1, nf))
                    emit_at.setdefault(slot, []).append(f)
                for j, f in enumerate(late_fillers):
                    nf = len(late_fillers)
                    slot = min(I - 1, int(I * 0.5) + (j * max(1, int(I * 0.45))) // max(1, nf))
                    emit_at.setdefault(slot, []).append(f)

                oo = {}
                pts = {}
                pending_pv = None
                tail_plan = None

                def sc_exp(hp, kti):
                    ct = hp
                    off = max(0, kti * 128 - q0)
                    st = psb.tile([128, 2, 512], F32, tag="st", name="st")
                    for hh in range(2):
                        nc.tensor.matmul(
                            st[:, hh, off:],
                            kt[64 * hh : 64 * hh + 64, ct, kti * 128 : (kti + 1) * 128],
                            qt[64 * hh : 64 * hh + 64, ct, q0 + off : q0 + 512],
                            start=True,
                            stop=True,
                            tile_position=(64 * hh, 0),
                        )
                    pt = ptp.tile([128, 2, 512], BF16, tag="pt", name="pt")
                    nc.scalar.activation(
                        pt[:, :, off:], st[:, :, off:], EXP, scale=0.125
                    )
                    if kti >= 4 * qc:
                        # diagonal: mask the 128-wide band; q' >= p keeps
                        nc.gpsimd.affine_select(
                            out=pt[:, :, off : off + 128],
                            in_=pt[:, :, off : off + 128],
                            compare_op=mybir.AluOpType.is_ge,
                            fill=0.0,
                            base=0,
                            pattern=[[0, 2], [1, 128]],
                            channel_multiplier=-1,
                        )
                    pts[(hp, kti)] = pt

                def emit_pv(hp, kti):
                    off = max(0, kti * 128 - q0)
                    pt = pts.pop((hp, kti))
                    for hh in range(2):
                        nc.tensor.matmul(
                            oo[hp][hh][0 : DH + 1, off:],
                            vp[:, kti, 2 * hp + hh, :],
                            pt[:, hh, off:],
                            start=(kti == 0),
                            stop=(kti == n_kt - 1),
                        )

                def normA(hp, hh, on_scalar=False):
                    def fn():
                        stg = normp.tile([96, 512], BF16, tag=f"stg{hh}", name="stg")
                        cp = nc.scalar.copy if on_scalar else nc.vector.tensor_copy
                        cp(stg[0 : DH + 1, :], oo[hp][hh][0 : DH + 1, :])
                        oo[hp][hh] = None
                        norm_state[(qc, hp, hh)] = stg
                    return fn

                def normB(hp, hh):
                    def fn():
                        stg = norm_state[(qc, hp, hh)]
                        zt = normp.tile([32, 512], BF16, tag=f"zt{hh}", name="zt")
                        nc.vector.transpose(zt[:], stg[64:96, :])
                        zv = zt[:].rearrange("p (a b) -> p a b", b=32)[:, :, 0]
                        with nc.allow_low_precision("bf16 softmax row-sum reciprocal"):
                            nc.vector.reciprocal(zv, zv)
                        zr = normp.tile([32, 512], BF16, tag=f"zr{hh}", name="zr")
                        nc.vector.transpose(zr[:], zt[:])
                        norm_state[(qc, hp, hh)] = (stg, zr)
                    return fn

                def normCD(hp, hh, tail=False):
                    def fn():
                        stg, zr = norm_state.pop((qc, hp, hh))
                        # broadcast 1/Z across 64 partitions with a K=1 matmul;
                        # the multiply follows immediately so the PSUM slot
                        # frees fast. At the tail the filler ring is held by
                        # in-flight out-projection accumulators, so draw from
                        # the freed PV-accumulator ring instead.
                        if tail:
                            rb = pso.tile([64, 512], F32, tag="oo", name="rb")
                        else:
                            rb = ppp.tile([64, 512], F32, tag="pp", name="rb")
                        nc.tensor.matmul(
                            rb[:], ones_bf[0:1, :], zr[0:1, :], start=True, stop=True
                        )
                        nc.vector.tensor_mul(
                            at[64 * hh : 64 * hh + 64, hp, q0 : q0 + 512],
                            stg[0:DH, :],
                            rb[:],
                        )
                    return fn

                for i, (hp, kti) in enumerate(iters):
                    if kti == 0:
                        oo[hp] = [
                            pso.tile([DH + 1, 512], F32, tag="oo", name="oo0"),
                            pso.tile([DH + 1, 512], F32, tag="oo", name="oo1"),
                        ]
                    sc_exp(hp, kti)
                    run_due()
                    for f in emit_at.get(i, []):
                        f()
                    if pending_pv is not None:
                        emit_pv(*pending_pv)
                    pending_pv = (hp, kti)
                    if kti == n_kt - 1:
                        emit_pv(*pending_pv)
                        pending_pv = None
                        if final and hp == 1:
                            tail_plan = {
                                "A0": normA(1, 0, on_scalar=True),
                                "A1": normA(1, 1, on_scalar=True),
                                "B0": normB(1, 0),
                                "B1": normB(1, 1),
                                "CD0": normCD(1, 0, tail=True),
                                "CD1": normCD(1, 1, tail=True),
                            }
                        else:
                            for hh in range(2):
                                defer(1 + hh, normA(hp, hh))
                                defer(2 + hh, normB(hp, hh))
                                defer(3 + hh, normCD(hp, hh))
                    tick[0] += 1
                return tail_plan

            norm_state = {}

            # ---- driver ----
            for u in p1_units(0):
                u()
            phase2(0, p1_units(1), [])
            phase2(1, p1_units(2), [])
            phase2(2, p1_units(3), p3_units(0))
            tail = phase2(3, [], p3_units(1) + p3_units(2), final=True)
            drain_deferred()
            # ---- hand-scheduled tail: final head-pair normalization
            # interleaved with the last out-projection blocks ----
            tail["A0"]()
            tail["A1"]()
            tail["B0"]()
            tail["B1"]()
            # qti 12-14 ct0 matmuls run while the reciprocal pipeline drains;
            # qti 13/14 borrow the idle 2-bank score-PSUM slots (one
            # accumulator per bank half)
            ps_pre = []
            for tt in range(3):
                blk = slice((12 + tt) * 128, (13 + tt) * 128)
                if tt == 0:
                    tiles = [
                        (nn, ppp.tile([128, 512], F32, tag="pp", name="ps_pre"))
                        for nn in range(2)
                    ]
                else:
                    st2 = psb.tile([128, 2, 512], F32, tag="st", name="ps_pre2")
                    tiles = [(nn, st2[:, nn, :]) for nn in range(2)]
                for nn, ps in tiles:
                    nc.tensor.matmul(
                        ps,
                        at[:, 0, blk],
                        wo[:, 0, nn * 512 : (nn + 1) * 512],
                        start=True,
                        stop=False,
                        skip_group_check=True,
                    )
                    ps_pre.append((blk, nn, ps))
            tail["CD0"]()
            tail["CD1"]()
            for blk, nn, ps in ps_pre:
                nc.tensor.matmul(
                    ps,
                    at[:, 1, blk],
                    wo[:, 1, nn * 512 : (nn + 1) * 512],
                    start=False,
                    stop=True,
                    skip_group_check=True,
                )
                ot = outp.tile([128, 512], BF16, tag="ot", name="ot12")
                nc.vector.tensor_copy(ot[:], ps)
                nc.sync.dma_start(out[blk, nn * 512 : (nn + 1) * 512], ot[:])
            unit_p3(3, 3, 0)()
            unit_p3(3, 3, 1)()
    nc.compile()
    return nc


def _get_nc():
    if "nc" not in _CACHE:
        _CACHE["nc"] = _build()
    return _CACHE["nc"]


def _in_maps(x, Wq, Wk, Wv, Wo):
    bf16 = ml_dtypes.bfloat16
    x = np.asarray(x, dtype=np.float32)
    xb = [
        np.ascontiguousarray(
            x[b].reshape(4, 512, 8, 128).transpose(3, 0, 2, 1)
        ).astype(bf16)
        for b in range(B)
    ]
    Wq = np.asarray(Wq, dtype=np.float32).astype(bf16)
    Wk = np.asarray(Wk, dtype=np.float32).astype(bf16)
    Wv = np.asarray(Wv, dtype=np.float32).astype(bf16)
    Wo = np.asarray(Wo, dtype=np.float32).astype(bf16)
    maps = []
    for core in range(N_CORES):
        b, g = divmod(core, 4)
        sl = slice(g * GC, (g + 1) * GC)
        # wq/wk[p, ct, dt, c] = W[sl][ct*128+c, dt*128+p]
        wqw = np.ascontiguousarray(
            Wq[sl].reshape(2, 128, 8, 128).transpose(3, 0, 2, 1)
        )
        wkw = np.ascontiguousarray(
            Wk[sl].reshape(2, 128, 8, 128).transpose(3, 0, 2, 1)
        )
        # wv[p, dt, c] = Wv[sl][c, dt*128+p]
        wvw = np.ascontiguousarray(Wv[sl].reshape(GC, 8, 128).transpose(2, 1, 0))
        # wo[p, ct, n] = Wo[n, g*256 + ct*128 + p]
        wow = np.ascontiguousarray(Wo[:, sl].reshape(D, 2, 128).transpose(2, 1, 0))
        maps.append(
            {
                "xs": xb[b],
                "wqs": wqw,
                "wks": wkw,
                "wvs": wvw,
                "wos": wow,
            }
        )
    return maps


def _run(x, Wq, Wk, Wv, Wo, **spmd_kwargs):
    nc = _get_nc()
    res = run_bass_kernel_spmd(
        nc, _in_maps(x, Wq, Wk, Wv, Wo), core_ids=list(range(N_CORES)), **spmd_kwargs
    )
    outs = [np.asarray(r["out"], dtype=np.float32) for r in res.results]
    full = np.stack(
        [
            outs[0] + outs[1] + outs[2] + outs[3],
            outs[4] + outs[5] + outs[6] + outs[7],
        ]
    ).astype(np.float32)
    return full, res


def kernel(x, Wq, Wk, Wv, Wo):
    full, _ = _run(x, Wq, Wk, Wv, Wo)
    return full
